# revision 15
# baseline (speedup 1.0000x reference)
"""GAT layer kernel for Trainium2, 8 NeuronCores, data-parallel over R=b*s.

Self-contained: takes full inputs, returns full output.

v3 design (per core, RC=6 replicas):
  - All-bf16 datapath (x cast host-side). Projection on PE: h_aug =
    x_r @ [W(c-major) | Ws | Wd]; h (bf16) plus per-node a_src scalars are
    written into ONE HBM row per node:
    row n = [h r0..r5 (6*256 bf16, c-major) | a_src 24 bf16 | pad] (3328 B).
    a_dst scalars stay in SBUF (asad [128, nt, r, 8]).
  - Edge phase chunked by dst-tile (128 dsts, dst-sorted slots padded to
    128-slot tiles). Per chunk ONE dma_gather fetches each edge slot's src
    row (wire-bound ~13.4us/chunk).
  - z = a_src[src] (gathered) + a_dst[dst] (PE expand via transposed one-hot)
    p = exp(leaky_relu(z)); den = segsum(p) (PE one-hot); denrec = 1/den
    (0.25 head-mean folded into the finalize STT).
  - msg = hg * p in-place (DVE 2x mode, bf16); num = segsum(msg) (PE one-hot,
    f32 PSUM, 3 replicas x 2 col-groups per half).
  - out = sum_h 0.25*denrec[d,h]*num[d,(c,h)] + bias, then DMA out.
  - Chunks are software-pipelined by emission order: gather(t+1) and
    eps(t+1) issue before chunk t's num matmuls; z/lr/exp/den/recip for
    t+1 issue before finalize(t) so no engine FIFO head-blocks.
  - DTW=128 so matmul weights are 128 wide (FWL-eligible); last dst tile
    covers 104 nodes.
"""

import math
import numpy as np
import ml_dtypes

B, S, N, F = 4, 12, 1000, 64
H, C = 4, 64
HC = H * C            # 256
R = B * S             # 48
NCORES = 8
RC = R // NCORES      # 6 replicas per core
NEG_SLOPE = 0.2
DTW = 128             # dst-tile width (7*128 + 104 cover N=1000)
NDT = 8
AC = RC * H           # 24 active scalar columns
ROWW = RC * HC + 128  # 1664 bf16 = 3328 B per h_hbm row (24 a_src + pad)

_CACHE = {}


def _tile_w(dt):
    return min(DTW, N - dt * DTW)   # 128 or 104 for the last tile


# --------------------------------------------------------------------------
# host-side index preprocessing
# --------------------------------------------------------------------------
def _prep_edges(edge_index):
    src0 = np.asarray(edge_index[0], dtype=np.int64)
    dst0 = np.asarray(edge_index[1], dtype=np.int64)
    keep = src0 != dst0                      # PyG remove_self_loops + NEG_INF mask
    s_all = np.concatenate([src0[keep], np.arange(N, dtype=np.int64)])
    d_all = np.concatenate([dst0[keep], np.arange(N, dtype=np.int64)])
    order = np.argsort(d_all, kind="stable")
    s_all, d_all = s_all[order], d_all[order]

    chunks = []
    for dt in range(NDT):
        lo = dt * DTW
        hi = lo + _tile_w(dt)
        m = (d_all >= lo) & (d_all < hi)
        ss, dd = s_all[m], d_all[m]
        cnt = len(ss)
        ntile = max(1, math.ceil(cnt / 128))
        pad = ntile * 128 - cnt
        ss = np.concatenate([ss, np.full(pad, 1000, np.int64)])   # pad -> row 1000
        dd = np.concatenate([dd, np.full(pad, lo, np.int64)])
        real = np.concatenate([np.ones(cnt, bool), np.zeros(pad, bool)])
        oh = np.zeros((128, ntile, DTW), np.float32)
        for j in range(ntile * 128):
            if real[j]:
                oh[j % 128, j // 128, dd[j] - lo] = 1.0
        chunks.append(dict(ntile=ntile, src=ss, oh=oh.astype(ml_dtypes.bfloat16),
                           ohT=np.ascontiguousarray(
                               oh.transpose(2, 1, 0)).astype(ml_dtypes.bfloat16)))

    maxt = max(c["ntile"] for c in chunks)
    T = sum(c["ntile"] for c in chunks)
    ihw = np.zeros((128, T * 8), np.int16)   # 128 slots = 8 idx columns
    oh_all = np.zeros((128, T, DTW), ml_dtypes.bfloat16)
    ohT_all = np.zeros((128, T, 128), ml_dtypes.bfloat16)
    t0 = 0
    for c in chunks:
        nt_, ss = c["ntile"], c["src"]
        ni = nt_ * 128
        a = np.zeros((16, ni // 16), np.int16)
        a[np.arange(ni) % 16, np.arange(ni) // 16] = ss.astype(np.int16)
        ihw[:, t0 * 8:(t0 + nt_) * 8] = np.tile(a, (8, 1))
        oh_all[:, t0:t0 + nt_, :] = c["oh"]
        ohT_all[:, t0:t0 + nt_, :] = c["ohT"]
        t0 += nt_
    return {
        "T": T, "maxt": maxt, "ntiles": [c["ntile"] for c in chunks],
        "oh": np.ascontiguousarray(oh_all.reshape(128, T * DTW)),
        "ohT": np.ascontiguousarray(ohT_all.reshape(128, T * 128)),
        "ih": ihw,
    }


def _prep_weights(W, att_src, att_dst):
    W = np.asarray(W, np.float32)
    Ws = np.zeros((F, H), np.float32)
    Wd = np.zeros((F, H), np.float32)
    for h in range(H):
        Ws[:, h] = W[:, h * C:(h + 1) * C] @ np.asarray(att_src, np.float32)[h]
        Wd[:, h] = W[:, h * C:(h + 1) * C] @ np.asarray(att_dst, np.float32)[h]
    # c-major head interleave: device col c*4+h = W col h*64+c
    Wc = np.empty_like(W)
    for h in range(H):
        Wc[:, np.arange(C) * H + h] = W[:, h * C:(h + 1) * C]
    return np.concatenate([Wc, Ws, Wd], axis=1).astype(ml_dtypes.bfloat16)


def _make_in_maps(x, W, att_src, att_dst, bias, ed):
    waug = _prep_weights(W, att_src, att_dst)
    bias_slab = np.tile(np.asarray(bias, np.float32)[None, :],
                        (128, RC)).reshape(128, RC * F)
    xr = np.ascontiguousarray(np.asarray(x, np.float32)).reshape(R, N, F)
    in_maps = []
    for cidx in range(NCORES):
        xc = xr[cidx * RC:(cidx + 1) * RC]
        xT = np.ascontiguousarray(
            xc.transpose(2, 0, 1).reshape(F, RC * N)).astype(ml_dtypes.bfloat16)
        in_maps.append({
            "xT": xT, "w_aug": waug, "oh": ed["oh"], "ohT": ed["ohT"],
            "ih": ed["ih"], "bias_slab": bias_slab,
        })
    return in_maps


# --------------------------------------------------------------------------
# device program
# --------------------------------------------------------------------------
def _build_program(ed):
    import concourse.bass as bass
    import concourse.mybir as mybir
    import concourse.tile as tile
    from concourse import bacc

    T, maxt = ed["T"], ed["maxt"]
    ntiles = ed["ntiles"]
    toff = np.concatenate([[0], np.cumsum(ntiles)]).astype(int)
    f32 = mybir.dt.float32
    bf16 = mybir.dt.bfloat16
    i16 = mybir.dt.int16
    Alu = mybir.AluOpType
    Act = mybir.ActivationFunctionType

    nc = bacc.Bacc("TRN2", target_bir_lowering=False, debug=False,
                   enable_asserts=False, num_devices=NCORES)

    xT_d = nc.dram_tensor("xT", [F, RC * N], bf16, kind="ExternalInput").ap()
    waug_d = nc.dram_tensor("w_aug", [F, 264], bf16, kind="ExternalInput").ap()
    oh_d = nc.dram_tensor("oh", [128, T * DTW], bf16, kind="ExternalInput").ap()
    ohT_d = nc.dram_tensor("ohT", [128, T * 128], bf16, kind="ExternalInput").ap()
    ih_d = nc.dram_tensor("ih", [128, T * 8], i16, kind="ExternalInput").ap()
    bias_d = nc.dram_tensor("bias_slab", [128, RC * F], f32, kind="ExternalInput").ap()
    out_d = nc.dram_tensor("out", [RC, N, F], f32, kind="ExternalOutput").ap()

    with tile.TileContext(nc) as tc:
        with (
            tc.tile_pool(name="const", bufs=1) as constp,
            tc.tile_pool(name="dram", bufs=1, space="DRAM") as dramp,
            tc.tile_pool(name="stage", bufs=3) as stagep,
            tc.tile_pool(name="edge", bufs=2) as edgep,
            tc.tile_pool(name="big", bufs=2) as bigp,
            tc.tile_pool(name="fin", bufs=2) as finp,
            tc.tile_pool(name="spsum", bufs=2, space="PSUM") as spsum,
            tc.tile_pool(name="npsum", bufs=2, space="PSUM") as npsum,
        ):
            h_hbm = dramp.tile([N + 1, ROWW], bf16)

            # ---- constants ----
            waug = constp.tile([F, 264], bf16)
            nc.sync.dma_start(waug[:], waug_d)
            oh = constp.tile([128, T, DTW], bf16)
            nc.sync.dma_start(oh[:], oh_d.rearrange("p (t d) -> p t d", d=DTW))
            ohT = constp.tile([128, T, 128], bf16)
            nc.sync.dma_start(ohT[:], ohT_d.rearrange("p (t e) -> p t e", e=128))
            ih = constp.tile([128, T * 8], i16)
            nc.sync.dma_start(ih[:], ih_d)
            bias_sl = constp.tile([128, RC, F], f32)
            nc.sync.dma_start(bias_sl[:], bias_d.rearrange("p (r f) -> p r f", f=F))

            # pad row 1000: h-part zeros, as-part -1000 => p == 0 for pad slots
            padrow = constp.tile([1, ROWW], bf16)
            nc.vector.memset(padrow[:], 0.0)
            nc.vector.memset(padrow[:, RC * HC:RC * HC + AC], -1000.0)
            nc.sync.dma_start(h_hbm[N:N + 1, :], padrow[:])

            # ---- phase A: projection; fills h_hbm + asad ----
            # asad[:, nt, r, 0:4] = a_src, [..., 4:8] = a_dst
            asad = constp.tile([128, NDT, RC, 8], bf16)
            nc.vector.memset(asad[:], 0.0)   # rows 104..127 of tile 7 stay 0
            for r in range(RC):
                xt = stagep.tile([F, N], bf16, tag="xt")
                nc.sync.dma_start(xt[:], xT_d[:, r * N:(r + 1) * N])
                hslab = stagep.tile([128, NDT, HC], bf16, tag="hslab")
                for nt in range(NDT):
                    n0 = nt * DTW
                    w = _tile_w(nt)
                    ps = spsum.tile([128, maxt * AC], f32, tag="scratch")
                    nc.tensor.matmul(out=ps[0:w, 0:264], lhsT=xt[:, n0:n0 + w],
                                     rhs=waug[:], start=True, stop=True)
                    nc.scalar.copy(out=hslab[0:w, nt, 0:128], in_=ps[0:w, 0:128])
                    nc.vector.tensor_copy(out=hslab[0:w, nt, 128:256],
                                          in_=ps[0:w, 128:256])
                    nc.vector.tensor_copy(out=asad[0:w, nt, r, :],
                                          in_=ps[0:w, 256:264])
                nc.sync.dma_start(
                    h_hbm[0:896, r * HC:(r + 1) * HC].rearrange(
                        "(a d) e -> d a e", d=DTW), hslab[:, 0:7, :])
                nc.sync.dma_start(h_hbm[896:1000, r * HC:(r + 1) * HC],
                                  hslab[0:104, 7, :])
            # a_src tail columns of each row: repack contiguous, then 2 DMAs
            as_sb = constp.tile([128, NDT, AC], bf16)
            nc.vector.tensor_copy(
                out=as_sb[:].rearrange("p n (r k) -> p n r k", k=4),
                in_=asad[:, :, :, 0:4])
            nc.sync.dma_start(
                h_hbm[0:896, RC * HC:RC * HC + AC].rearrange(
                    "(a d) e -> d a e", d=DTW), as_sb[:, 0:7, :])
            nc.sync.dma_start(h_hbm[896:1000, RC * HC:RC * HC + AC],
                              as_sb[0:104, 7, :])

            # ---- edge phase, software-pipelined over the 8 dst-tile chunks --
            def gather(t):
                nt_ = ntiles[t]
                ni = nt_ * 128
                assert ni <= 2032, "gather exceeds SWDGE FIFO"
                hg = bigp.tile([128, maxt, ROWW], bf16, tag="big")
                nc.gpsimd.dma_gather(
                    out_ap=hg[:, 0:nt_, :], in_ap=h_hbm[:],
                    idxs_ap=ih[:, toff[t] * 8:(toff[t] + nt_) * 8],
                    num_idxs=ni, num_idxs_reg=ni, elem_size=ROWW,
                    single_packet=False)
                return hg

            def eps_mm(t):
                nt_ = ntiles[t]
                eps = spsum.tile([128, maxt * AC], f32, tag="scratch")
                for k in range(nt_):
                    nc.tensor.matmul(out=eps[:, k * AC:(k + 1) * AC],
                                     lhsT=ohT[:, toff[t] + k, :],
                                     rhs=asad[:, t, :, 4:8], start=True, stop=True)
                return eps

            def attn_prep(t, hg, eps):
                nt_ = ntiles[t]
                z = edgep.tile([128, maxt, AC], bf16, tag="z")
                nc.vector.tensor_tensor(
                    out=z[:, 0:nt_, :],
                    in0=hg[:, 0:nt_, RC * HC:RC * HC + AC],
                    in1=eps[:, 0:nt_ * AC].rearrange("p (t a) -> p t a", a=AC),
                    op=Alu.add)
                nc.vector.scalar_tensor_tensor(
                    out=z[:, 0:nt_, :], in0=z[:, 0:nt_, :], scalar=NEG_SLOPE,
                    in1=z[:, 0:nt_, :], op0=Alu.mult, op1=Alu.max)
                p_bf = edgep.tile([128, maxt, AC], bf16, tag="p")
                nc.scalar.activation(out=p_bf[:, 0:nt_, :], in_=z[:, 0:nt_, :],
                                     func=Act.Exp)
                return p_bf

            def den_mm(t, p_bf):
                nt_ = ntiles[t]
                den_ps = spsum.tile([128, maxt * AC], f32, tag="scratch")
                for k in range(nt_):
                    nc.tensor.matmul(out=den_ps[:, 0:AC],
                                     lhsT=oh[:, toff[t] + k, :],
                                     rhs=p_bf[:, k, :],
                                     start=(k == 0), stop=(k == nt_ - 1))
                denrec = stagep.tile([128, AC], f32, tag="denrec")
                nc.vector.reciprocal(out=denrec[:], in_=den_ps[:, 0:AC])
                nc.vector.tensor_scalar_mul(denrec[:], denrec[:], 0.25)
                return denrec

            def mult_msg(t, hg, p_bf):
                nt_ = ntiles[t]
                for r in range(RC):
                    hgr = hg[:, 0:nt_, r * HC:(r + 1) * HC].rearrange(
                        "p t (c h) -> p t c h", h=H)
                    pb = p_bf[:, 0:nt_, 4 * r:4 * r + 4].rearrange(
                        "p t (o h) -> p t o h", o=1).to_broadcast(
                        [128, nt_, C, H])
                    nc.vector.tensor_tensor(out=hgr, in0=hgr, in1=pb,
                                            op=Alu.mult)

            def num_mm(t, hg):
                nt_ = ntiles[t]
                nps = npsum.tile([128, RC * HC], f32, tag="num")
                for k in range(nt_):
                    lhsT = oh[:, toff[t] + k, :]
                    for g in range(3):
                        nc.tensor.matmul(out=nps[:, g * 512:(g + 1) * 512],
                                         lhsT=lhsT,
                                         rhs=hg[:, k, g * 512:(g + 1) * 512],
                                         start=(k == 0), stop=(k == nt_ - 1))
                return nps

            def finalize(t, nps, denrec):
                n0 = t * DTW
                w = _tile_w(t)
                for half in range(2):
                    c0 = half * (RC * HC // 2)
                    numn = finp.tile([128, RC // 2, HC], f32, tag="numn")
                    drb = denrec[:, half * 12:half * 12 + 12].rearrange(
                        "d (r o h) -> d r o h", h=H, o=1).to_broadcast(
                        [128, RC // 2, C, H])
                    nc.vector.tensor_tensor(
                        out=numn[:].rearrange("d r (c h) -> d r c h", h=H),
                        in0=nps[:, c0:c0 + 768].rearrange(
                            "d (r c h) -> d r c h", h=H, c=C),
                        in1=drb, op=Alu.mult)
                    n4 = numn[:].rearrange("d r (c h) -> d r c h", h=H)
                    t1 = finp.tile([128, RC // 2, C], f32, tag="t1")
                    t2 = finp.tile([128, RC // 2, C], f32, tag="t2")
                    ob = finp.tile([128, RC // 2, C], f32, tag="ob")
                    nc.vector.tensor_tensor(out=t1[:], in0=n4[:, :, :, 0],
                                            in1=n4[:, :, :, 1], op=Alu.add)
                    nc.vector.tensor_tensor(out=t2[:], in0=n4[:, :, :, 2],
                                            in1=n4[:, :, :, 3], op=Alu.add)
                    nc.vector.tensor_tensor(out=t1[:], in0=t1[:], in1=t2[:],
                                            op=Alu.add)
                    nc.vector.tensor_tensor(
                        out=ob[:], in0=t1[:],
                        in1=bias_sl[:, half * 3:half * 3 + 3, :], op=Alu.add)
                    nc.sync.dma_start(
                        out_d[half * 3:half * 3 + 3, n0:n0 + w, :].rearrange(
                            "r d f -> d r f"), ob[0:w])

            # prologue
            hgs = {0: gather(0)}
            epss = {0: eps_mm(0)}
            p0 = attn_prep(0, hgs[0], epss[0])
            pbs = {0: p0}
            drs = {0: den_mm(0, p0)}
            for t in range(NDT):
                if t + 1 < NDT:
                    hgs[t + 1] = gather(t + 1)
                    epss[t + 1] = eps_mm(t + 1)
                mult_msg(t, hgs[t], pbs[t])
                nps = num_mm(t, hgs[t])
                if t + 1 < NDT:
                    pbs[t + 1] = attn_prep(t + 1, hgs[t + 1], epss[t + 1])
                finalize(t, nps, drs[t])
                if t + 1 < NDT:
                    drs[t + 1] = den_mm(t + 1, pbs[t + 1])

    nc.compile()
    return nc


# --------------------------------------------------------------------------
# public entry point
# --------------------------------------------------------------------------
def kernel(x, edge_index, W, att_src, att_dst, bias):
    key = hash(np.asarray(edge_index).tobytes())
    if key not in _CACHE:
        ed = _prep_edges(edge_index)
        _CACHE[key] = (_build_program(ed), ed)
    nc, ed = _CACHE[key]

    in_maps = _make_in_maps(x, W, att_src, att_dst, bias, ed)
    from concourse import bass_utils
    res = bass_utils.run_bass_kernel_spmd(nc, in_maps, core_ids=list(range(NCORES)))
    outs = [res.results[c]["out"] for c in range(NCORES)]
    out = np.concatenate(outs, axis=0).reshape(B, S, N, F).astype(np.float32)
    return out


# revision 21
# speedup vs baseline: 1.0465x; 1.0465x over previous
"""GAT layer kernel for Trainium2, 8 NeuronCores, data-parallel over R=b*s.

Self-contained: takes full inputs, returns full output.

v3 design (per core, RC=6 replicas):
  - All-bf16 datapath (x cast host-side). Projection on PE: h_aug =
    x_r @ [W(c-major) | Ws | Wd]; h (bf16) plus per-node a_src scalars are
    written into ONE HBM row per node:
    row n = [h r0..r5 (6*256 bf16, c-major) | a_src 24 bf16 | pad] (3328 B).
    a_dst scalars stay in SBUF (asad [128, nt, r, 8]).
  - Edge phase chunked by dst-tile (128 dsts, dst-sorted slots padded to
    128-slot tiles). Per chunk ONE dma_gather fetches each edge slot's src
    row (wire-bound ~13.4us/chunk).
  - z = a_src[src] (gathered) + a_dst[dst] (PE expand via transposed one-hot)
    p = exp(leaky_relu(z)); den = segsum(p) (PE one-hot); denrec = 1/den
    (0.25 head-mean folded into the finalize STT).
  - msg = hg * p in-place (DVE 2x mode, bf16); num = segsum(msg) (PE one-hot,
    f32 PSUM, 3 replicas x 2 col-groups per half).
  - out = sum_h 0.25*denrec[d,h]*num[d,(c,h)] + bias, then DMA out.
  - Chunks are software-pipelined by emission order: gather(t+1) and
    eps(t+1) issue before chunk t's num matmuls; z/lr/exp/den/recip for
    t+1 issue before finalize(t) so no engine FIFO head-blocks.
  - DTW=128 so matmul weights are 128 wide (FWL-eligible); last dst tile
    covers 104 nodes.
"""

import math
import numpy as np
import ml_dtypes

B, S, N, F = 4, 12, 1000, 64
H, C = 4, 64
HC = H * C            # 256
R = B * S             # 48
NCORES = 8
RC = R // NCORES      # 6 replicas per core
NEG_SLOPE = 0.2
DTW = 128             # dst-tile width (7*128 + 104 cover N=1000)
NDT = 8
AC = RC * H           # 24 active scalar columns
ROWW = RC * HC + 128  # 1664 bf16 = 3328 B per h_hbm row (24 a_src + pad)

_CACHE = {}


def _tile_w(dt):
    return min(DTW, N - dt * DTW)   # 128 or 104 for the last tile


# --------------------------------------------------------------------------
# host-side index preprocessing
# --------------------------------------------------------------------------
def _prep_edges(edge_index):
    src0 = np.asarray(edge_index[0], dtype=np.int64)
    dst0 = np.asarray(edge_index[1], dtype=np.int64)
    keep = src0 != dst0                      # PyG remove_self_loops + NEG_INF mask
    s_all = np.concatenate([src0[keep], np.arange(N, dtype=np.int64)])
    d_all = np.concatenate([dst0[keep], np.arange(N, dtype=np.int64)])
    order = np.argsort(d_all, kind="stable")
    s_all, d_all = s_all[order], d_all[order]

    chunks = []
    for dt in range(NDT):
        lo = dt * DTW
        hi = lo + _tile_w(dt)
        m = (d_all >= lo) & (d_all < hi)
        ss, dd = s_all[m], d_all[m]
        cnt = len(ss)
        ntile = max(1, math.ceil(cnt / 128))
        pad = ntile * 128 - cnt
        ss = np.concatenate([ss, np.full(pad, 1000, np.int64)])   # pad -> row 1000
        dd = np.concatenate([dd, np.full(pad, lo, np.int64)])
        real = np.concatenate([np.ones(cnt, bool), np.zeros(pad, bool)])
        oh = np.zeros((128, ntile, DTW), np.float32)
        for j in range(ntile * 128):
            if real[j]:
                oh[j % 128, j // 128, dd[j] - lo] = 1.0
        chunks.append(dict(ntile=ntile, src=ss, oh=oh.astype(ml_dtypes.bfloat16),
                           ohT=np.ascontiguousarray(
                               oh.transpose(2, 1, 0)).astype(ml_dtypes.bfloat16)))

    maxt = max(c["ntile"] for c in chunks)
    T = sum(c["ntile"] for c in chunks)
    ihw = np.zeros((128, T * 8), np.int16)   # 128 slots = 8 idx columns
    oh_all = np.zeros((128, T, DTW), ml_dtypes.bfloat16)
    ohT_all = np.zeros((128, T, 128), ml_dtypes.bfloat16)
    t0 = 0
    for c in chunks:
        nt_, ss = c["ntile"], c["src"]
        ni = nt_ * 128
        a = np.zeros((16, ni // 16), np.int16)
        a[np.arange(ni) % 16, np.arange(ni) // 16] = ss.astype(np.int16)
        ihw[:, t0 * 8:(t0 + nt_) * 8] = np.tile(a, (8, 1))
        oh_all[:, t0:t0 + nt_, :] = c["oh"]
        ohT_all[:, t0:t0 + nt_, :] = c["ohT"]
        t0 += nt_
    return {
        "T": T, "maxt": maxt, "ntiles": [c["ntile"] for c in chunks],
        "oh": np.ascontiguousarray(oh_all.reshape(128, T * DTW)),
        "ohT": np.ascontiguousarray(ohT_all.reshape(128, T * 128)),
        "ih": ihw,
    }


def _prep_weights(W, att_src, att_dst):
    W = np.asarray(W, np.float32)
    Ws = np.zeros((F, H), np.float32)
    Wd = np.zeros((F, H), np.float32)
    for h in range(H):
        Ws[:, h] = W[:, h * C:(h + 1) * C] @ np.asarray(att_src, np.float32)[h]
        Wd[:, h] = W[:, h * C:(h + 1) * C] @ np.asarray(att_dst, np.float32)[h]
    # c-major head interleave: device col c*4+h = W col h*64+c
    Wc = np.empty_like(W)
    for h in range(H):
        Wc[:, np.arange(C) * H + h] = W[:, h * C:(h + 1) * C]
    return np.concatenate([Wc, Ws, Wd], axis=1).astype(ml_dtypes.bfloat16)


def _make_in_maps(x, W, att_src, att_dst, bias, ed):
    waug = _prep_weights(W, att_src, att_dst)
    bias_slab = np.tile(np.asarray(bias, np.float32)[None, :],
                        (128, RC)).reshape(128, RC * F)
    xr = np.ascontiguousarray(np.asarray(x, np.float32)).reshape(R, N, F)
    in_maps = []
    for cidx in range(NCORES):
        xc = xr[cidx * RC:(cidx + 1) * RC]
        xT = np.ascontiguousarray(
            xc.transpose(2, 0, 1).reshape(F, RC * N)).astype(ml_dtypes.bfloat16)
        in_maps.append({
            "xT": xT, "w_aug": waug, "oh": ed["oh"], "ohT": ed["ohT"],
            "ih": ed["ih"], "bias_slab": bias_slab,
        })
    return in_maps


# --------------------------------------------------------------------------
# device program
# --------------------------------------------------------------------------
def _build_program(ed):
    import concourse.bass as bass
    import concourse.mybir as mybir
    import concourse.tile as tile
    from concourse import bacc

    T, maxt = ed["T"], ed["maxt"]
    ntiles = ed["ntiles"]
    toff = np.concatenate([[0], np.cumsum(ntiles)]).astype(int)
    f32 = mybir.dt.float32
    bf16 = mybir.dt.bfloat16
    i16 = mybir.dt.int16
    Alu = mybir.AluOpType
    Act = mybir.ActivationFunctionType

    nc = bacc.Bacc("TRN2", target_bir_lowering=False, debug=False,
                   enable_asserts=False, num_devices=NCORES)

    xT_d = nc.dram_tensor("xT", [F, RC * N], bf16, kind="ExternalInput").ap()
    waug_d = nc.dram_tensor("w_aug", [F, 264], bf16, kind="ExternalInput").ap()
    oh_d = nc.dram_tensor("oh", [128, T * DTW], bf16, kind="ExternalInput").ap()
    ohT_d = nc.dram_tensor("ohT", [128, T * 128], bf16, kind="ExternalInput").ap()
    ih_d = nc.dram_tensor("ih", [128, T * 8], i16, kind="ExternalInput").ap()
    bias_d = nc.dram_tensor("bias_slab", [128, RC * F], f32, kind="ExternalInput").ap()
    out_d = nc.dram_tensor("out", [RC, N, F], f32, kind="ExternalOutput").ap()

    with tile.TileContext(nc) as tc:
        with (
            tc.tile_pool(name="const", bufs=1) as constp,
            tc.tile_pool(name="dram", bufs=1, space="DRAM") as dramp,
            tc.tile_pool(name="stage", bufs=3) as stagep,
            tc.tile_pool(name="edge", bufs=2) as edgep,
            tc.tile_pool(name="big", bufs=2) as bigp,
            tc.tile_pool(name="fin", bufs=2) as finp,
            tc.tile_pool(name="spsum", bufs=2, space="PSUM") as spsum,
            tc.tile_pool(name="npsum", bufs=2, space="PSUM") as npsum,
        ):
            h_hbm = dramp.tile([N + 1, ROWW], bf16)

            # ---- constants needed before/during phase A ----
            waug = constp.tile([F, 264], bf16)
            nc.sync.dma_start(waug[:], waug_d)
            ih = constp.tile([128, T * 8], i16)
            nc.sync.dma_start(ih[:], ih_d)

            # pad row 1000: h-part zeros, as-part -1000 => p == 0 for pad slots
            padrow = constp.tile([1, ROWW], bf16)
            nc.vector.memset(padrow[:], 0.0)
            nc.vector.memset(padrow[:, RC * HC:RC * HC + AC], -1000.0)
            nc.sync.dma_start(h_hbm[N:N + 1, :], padrow[:])

            # ---- phase A: projection; fills h_hbm + asad ----
            # asad[:, nt, r, 0:4] = a_src, [..., 4:8] = a_dst
            asad = constp.tile([128, NDT, RC, 8], bf16)
            nc.vector.memset(asad[:], 0.0)   # rows 104..127 of tile 7 stay 0
            for r in range(RC):
                xt = stagep.tile([F, N], bf16, tag="xt")
                nc.sync.dma_start(xt[:], xT_d[:, r * N:(r + 1) * N])
                hslab = stagep.tile([128, NDT, HC], bf16, tag="hslab")
                for nt in range(NDT):
                    n0 = nt * DTW
                    w = _tile_w(nt)
                    ps = spsum.tile([128, maxt * AC], f32, tag="scratch")
                    nc.tensor.matmul(out=ps[0:w, 0:264], lhsT=xt[:, n0:n0 + w],
                                     rhs=waug[:], start=True, stop=True)
                    nc.scalar.copy(out=hslab[0:w, nt, 0:128], in_=ps[0:w, 0:128])
                    nc.vector.tensor_copy(out=hslab[0:w, nt, 128:256],
                                          in_=ps[0:w, 128:256])
                    nc.vector.tensor_copy(out=asad[0:w, nt, r, :],
                                          in_=ps[0:w, 256:264])
                nc.sync.dma_start(
                    h_hbm[0:896, r * HC:(r + 1) * HC].rearrange(
                        "(a d) e -> d a e", d=DTW), hslab[:, 0:7, :])
                nc.sync.dma_start(h_hbm[896:1000, r * HC:(r + 1) * HC],
                                  hslab[0:104, 7, :])
            # a_src tail columns of each row: repack contiguous, then 2 DMAs
            as_sb = constp.tile([128, NDT, AC], bf16)
            nc.vector.tensor_copy(
                out=as_sb[:].rearrange("p n (r k) -> p n r k", k=4),
                in_=asad[:, :, :, 0:4])
            nc.sync.dma_start(
                h_hbm[0:896, RC * HC:RC * HC + AC].rearrange(
                    "(a d) e -> d a e", d=DTW), as_sb[:, 0:7, :])
            nc.sync.dma_start(h_hbm[896:1000, RC * HC:RC * HC + AC],
                              as_sb[0:104, 7, :])

            # ---- edge-phase constants (after phase A's DMAs in queue order)
            ohT = constp.tile([128, T, 128], bf16)
            nc.sync.dma_start(ohT[:], ohT_d.rearrange("p (t e) -> p t e", e=128))
            oh = constp.tile([128, T, DTW], bf16)
            nc.sync.dma_start(oh[:], oh_d.rearrange("p (t d) -> p t d", d=DTW))
            bias_sl = constp.tile([128, RC, F], f32)
            nc.sync.dma_start(bias_sl[:], bias_d.rearrange("p (r f) -> p r f", f=F))

            # ---- edge phase, software-pipelined over the 8 dst-tile chunks --
            def gather(t):
                nt_ = ntiles[t]
                ni = nt_ * 128
                assert ni <= 2032, "gather exceeds SWDGE FIFO"
                hg = bigp.tile([128, maxt, ROWW], bf16, tag="big")
                nc.gpsimd.dma_gather(
                    out_ap=hg[:, 0:nt_, :], in_ap=h_hbm[:],
                    idxs_ap=ih[:, toff[t] * 8:(toff[t] + nt_) * 8],
                    num_idxs=ni, num_idxs_reg=ni, elem_size=ROWW,
                    single_packet=False)
                return hg

            def eps_mm(t):
                nt_ = ntiles[t]
                eps = spsum.tile([128, maxt * AC], f32, tag="scratch")
                for k in range(nt_):
                    nc.tensor.matmul(out=eps[:, k * AC:(k + 1) * AC],
                                     lhsT=ohT[:, toff[t] + k, :],
                                     rhs=asad[:, t, :, 4:8], start=True, stop=True)
                return eps

            def attn_prep(t, hg, eps):
                nt_ = ntiles[t]
                z = edgep.tile([128, maxt, AC], bf16, tag="z")
                nc.vector.tensor_tensor(
                    out=z[:, 0:nt_, :],
                    in0=hg[:, 0:nt_, RC * HC:RC * HC + AC],
                    in1=eps[:, 0:nt_ * AC].rearrange("p (t a) -> p t a", a=AC),
                    op=Alu.add)
                nc.vector.scalar_tensor_tensor(
                    out=z[:, 0:nt_, :], in0=z[:, 0:nt_, :], scalar=NEG_SLOPE,
                    in1=z[:, 0:nt_, :], op0=Alu.mult, op1=Alu.max)
                p_bf = edgep.tile([128, maxt, AC], bf16, tag="p")
                nc.scalar.activation(out=p_bf[:, 0:nt_, :], in_=z[:, 0:nt_, :],
                                     func=Act.Exp)
                return p_bf

            def den_mm_pe(t, p_bf):
                nt_ = ntiles[t]
                den_ps = spsum.tile([128, maxt * AC], f32, tag="scratch")
                for k in range(nt_):
                    nc.tensor.matmul(out=den_ps[:, 0:AC],
                                     lhsT=oh[:, toff[t] + k, :],
                                     rhs=p_bf[:, k, :],
                                     start=(k == 0), stop=(k == nt_ - 1))
                return den_ps

            def den_recip(den_ps):
                denrec = stagep.tile([128, AC], f32, tag="denrec")
                nc.vector.reciprocal(out=denrec[:], in_=den_ps[:, 0:AC])
                nc.vector.tensor_scalar_mul(denrec[:], denrec[:], 0.25)
                return denrec

            def mult_msg(t, hg, p_bf):
                nt_ = ntiles[t]
                for r in range(RC):
                    hgr = hg[:, 0:nt_, r * HC:(r + 1) * HC].rearrange(
                        "p t (c h) -> p t c h", h=H)
                    pb = p_bf[:, 0:nt_, 4 * r:4 * r + 4].rearrange(
                        "p t (o h) -> p t o h", o=1).to_broadcast(
                        [128, nt_, C, H])
                    nc.vector.tensor_tensor(out=hgr, in0=hgr, in1=pb,
                                            op=Alu.mult)

            def num_mm(t, hg):
                nt_ = ntiles[t]
                nps = npsum.tile([128, RC * HC], f32, tag="num")
                for k in range(nt_):
                    lhsT = oh[:, toff[t] + k, :]
                    for g in range(3):
                        nc.tensor.matmul(out=nps[:, g * 512:(g + 1) * 512],
                                         lhsT=lhsT,
                                         rhs=hg[:, k, g * 512:(g + 1) * 512],
                                         start=(k == 0), stop=(k == nt_ - 1))
                return nps

            def finalize(t, nps, denrec):
                n0 = t * DTW
                w = _tile_w(t)
                for half in range(2):
                    c0 = half * (RC * HC // 2)
                    numn = finp.tile([128, RC // 2, HC], f32, tag="numn")
                    drb = denrec[:, half * 12:half * 12 + 12].rearrange(
                        "d (r o h) -> d r o h", h=H, o=1).to_broadcast(
                        [128, RC // 2, C, H])
                    nc.vector.tensor_tensor(
                        out=numn[:].rearrange("d r (c h) -> d r c h", h=H),
                        in0=nps[:, c0:c0 + 768].rearrange(
                            "d (r c h) -> d r c h", h=H, c=C),
                        in1=drb, op=Alu.mult)
                    n4 = numn[:].rearrange("d r (c h) -> d r c h", h=H)
                    t1 = finp.tile([128, RC // 2, C], f32, tag="t1")
                    t2 = finp.tile([128, RC // 2, C], f32, tag="t2")
                    ob = finp.tile([128, RC // 2, C], f32, tag="ob")
                    nc.vector.tensor_tensor(out=t1[:], in0=n4[:, :, :, 0],
                                            in1=n4[:, :, :, 1], op=Alu.add)
                    nc.vector.tensor_tensor(out=t2[:], in0=n4[:, :, :, 2],
                                            in1=n4[:, :, :, 3], op=Alu.add)
                    nc.vector.tensor_tensor(out=t1[:], in0=t1[:], in1=t2[:],
                                            op=Alu.add)
                    nc.vector.tensor_tensor(
                        out=ob[:], in0=t1[:],
                        in1=bias_sl[:, half * 3:half * 3 + 3, :], op=Alu.add)
                    nc.sync.dma_start(
                        out_d[half * 3:half * 3 + 3, n0:n0 + w, :].rearrange(
                            "r d f -> d r f"), ob[0:w])

            # prologue: chunk 0 attn-prep, plus chunk 1 gather/eps in flight
            hgs, epss, pbs, dps = {}, {}, {}, {}
            hgs[0] = gather(0)
            epss[0] = eps_mm(0)
            pbs[0] = attn_prep(0, hgs[0], epss[0])
            dps[0] = den_mm_pe(0, pbs[0])
            hgs[1] = gather(1)
            epss[1] = eps_mm(1)
            for t in range(NDT):
                denrec = den_recip(dps[t])
                mult_msg(t, hgs[t], pbs[t])
                nps = num_mm(t, hgs[t])
                if t + 1 < NDT:
                    pbs[t + 1] = attn_prep(t + 1, hgs[t + 1], epss[t + 1])
                    dps[t + 1] = den_mm_pe(t + 1, pbs[t + 1])
                if t + 2 < NDT:
                    hgs[t + 2] = gather(t + 2)
                    epss[t + 2] = eps_mm(t + 2)
                finalize(t, nps, denrec)

    nc.compile()
    return nc


# --------------------------------------------------------------------------
# public entry point
# --------------------------------------------------------------------------
def kernel(x, edge_index, W, att_src, att_dst, bias):
    key = hash(np.asarray(edge_index).tobytes())
    if key not in _CACHE:
        ed = _prep_edges(edge_index)
        _CACHE[key] = (_build_program(ed), ed)
    nc, ed = _CACHE[key]

    in_maps = _make_in_maps(x, W, att_src, att_dst, bias, ed)
    from concourse import bass_utils
    res = bass_utils.run_bass_kernel_spmd(nc, in_maps, core_ids=list(range(NCORES)))
    outs = [res.results[c]["out"] for c in range(NCORES)]
    out = np.concatenate(outs, axis=0).reshape(B, S, N, F).astype(np.float32)
    return out


# revision 26
# speedup vs baseline: 1.0582x; 1.0111x over previous
"""GAT layer kernel for Trainium2, 8 NeuronCores, data-parallel over R=b*s.

Self-contained: takes full inputs, returns full output.

v3 design (per core, RC=6 replicas):
  - All-bf16 datapath (x cast host-side). Projection on PE: h_aug =
    x_r @ [W(c-major) | Ws | Wd]; h (bf16) plus per-node a_src scalars are
    written into ONE HBM row per node:
    row n = [h r0..r5 (6*256 bf16, c-major) | a_src 24 bf16 | pad] (3328 B).
    a_dst scalars stay in SBUF (asad [128, nt, r, 8]).
  - Edge phase chunked by dst-tile (128 dsts, dst-sorted slots padded to
    128-slot tiles). Per chunk ONE dma_gather fetches each edge slot's src
    row (wire-bound ~13.4us/chunk).
  - z = a_src[src] (gathered) + a_dst[dst] (PE expand via transposed one-hot)
    p = exp(leaky_relu(z)); den = segsum(p) (PE one-hot); denrec = 1/den
    (0.25 head-mean folded into the finalize STT).
  - msg = hg * p in-place (DVE 2x mode, bf16); num = segsum(msg) (PE one-hot,
    f32 PSUM, 3 replicas x 2 col-groups per half).
  - out = sum_h 0.25*denrec[d,h]*num[d,(c,h)] + bias, then DMA out.
  - Chunks are software-pipelined by emission order: gather(t+1) and
    eps(t+1) issue before chunk t's num matmuls; z/lr/exp/den/recip for
    t+1 issue before finalize(t) so no engine FIFO head-blocks.
  - DTW=128 so matmul weights are 128 wide (FWL-eligible); last dst tile
    covers 104 nodes.
"""

import math
import numpy as np
import ml_dtypes

B, S, N, F = 4, 12, 1000, 64
H, C = 4, 64
HC = H * C            # 256
R = B * S             # 48
NCORES = 8
RC = R // NCORES      # 6 replicas per core
NEG_SLOPE = 0.2
DTW = 128             # dst-tile width (7*128 + 104 cover N=1000)
NDT = 8
AC = RC * H           # 24 active scalar columns
ROWW = RC * HC + 128  # 1664 bf16 = 3328 B per h_hbm row (24 a_src + pad)

_CACHE = {}


def _tile_w(dt):
    return min(DTW, N - dt * DTW)   # 128 or 104 for the last tile


# --------------------------------------------------------------------------
# host-side index preprocessing
# --------------------------------------------------------------------------
def _prep_edges(edge_index):
    src0 = np.asarray(edge_index[0], dtype=np.int64)
    dst0 = np.asarray(edge_index[1], dtype=np.int64)
    keep = src0 != dst0                      # PyG remove_self_loops + NEG_INF mask
    s_all = np.concatenate([src0[keep], np.arange(N, dtype=np.int64)])
    d_all = np.concatenate([dst0[keep], np.arange(N, dtype=np.int64)])
    order = np.argsort(d_all, kind="stable")
    s_all, d_all = s_all[order], d_all[order]

    chunks = []
    for dt in range(NDT):
        lo = dt * DTW
        hi = lo + _tile_w(dt)
        m = (d_all >= lo) & (d_all < hi)
        ss, dd = s_all[m], d_all[m]
        cnt = len(ss)
        ntile = max(1, math.ceil(cnt / 128))
        pad = ntile * 128 - cnt
        ss = np.concatenate([ss, np.full(pad, 1000, np.int64)])   # pad -> row 1000
        dd = np.concatenate([dd, np.full(pad, lo, np.int64)])
        real = np.concatenate([np.ones(cnt, bool), np.zeros(pad, bool)])
        oh = np.zeros((128, ntile, DTW), np.float32)
        for j in range(ntile * 128):
            if real[j]:
                oh[j % 128, j // 128, dd[j] - lo] = 1.0
        chunks.append(dict(ntile=ntile, src=ss, oh=oh.astype(ml_dtypes.bfloat16),
                           ohT=np.ascontiguousarray(
                               oh.transpose(2, 1, 0)).astype(ml_dtypes.bfloat16)))

    maxt = max(c["ntile"] for c in chunks)
    T = sum(c["ntile"] for c in chunks)
    ihw = np.zeros((128, T * 8), np.int16)   # 128 slots = 8 idx columns
    oh_all = np.zeros((128, T, DTW), ml_dtypes.bfloat16)
    ohT_all = np.zeros((128, T, 128), ml_dtypes.bfloat16)
    t0 = 0
    for c in chunks:
        nt_, ss = c["ntile"], c["src"]
        ni = nt_ * 128
        a = np.zeros((16, ni // 16), np.int16)
        a[np.arange(ni) % 16, np.arange(ni) // 16] = ss.astype(np.int16)
        ihw[:, t0 * 8:(t0 + nt_) * 8] = np.tile(a, (8, 1))
        oh_all[:, t0:t0 + nt_, :] = c["oh"]
        ohT_all[:, t0:t0 + nt_, :] = c["ohT"]
        t0 += nt_
    return {
        "T": T, "maxt": maxt, "ntiles": [c["ntile"] for c in chunks],
        "oh": np.ascontiguousarray(oh_all.reshape(128, T * DTW)),
        "ohT": np.ascontiguousarray(ohT_all.reshape(128, T * 128)),
        "ih": ihw,
    }


def _prep_weights(W, att_src, att_dst):
    W = np.asarray(W, np.float32)
    Ws = np.zeros((F, H), np.float32)
    Wd = np.zeros((F, H), np.float32)
    for h in range(H):
        Ws[:, h] = W[:, h * C:(h + 1) * C] @ np.asarray(att_src, np.float32)[h]
        Wd[:, h] = W[:, h * C:(h + 1) * C] @ np.asarray(att_dst, np.float32)[h]
    # c-major head interleave: device col c*4+h = W col h*64+c
    Wc = np.empty_like(W)
    for h in range(H):
        Wc[:, np.arange(C) * H + h] = W[:, h * C:(h + 1) * C]
    return np.concatenate([Wc, Ws, Wd], axis=1).astype(ml_dtypes.bfloat16)


def _make_in_maps(x, W, att_src, att_dst, bias, ed):
    waug = _prep_weights(W, att_src, att_dst)
    bias_slab = np.tile(np.asarray(bias, np.float32)[None, :],
                        (128, RC)).reshape(128, RC * F)
    xr = np.ascontiguousarray(np.asarray(x, np.float32)).reshape(R, N, F)
    in_maps = []
    for cidx in range(NCORES):
        xc = xr[cidx * RC:(cidx + 1) * RC]
        xT = np.ascontiguousarray(
            xc.transpose(2, 0, 1).reshape(F, RC * N)).astype(ml_dtypes.bfloat16)
        in_maps.append({
            "xT": xT, "w_aug": waug, "oh": ed["oh"], "ohT": ed["ohT"],
            "ih": ed["ih"], "bias_slab": bias_slab,
        })
    return in_maps


# --------------------------------------------------------------------------
# device program
# --------------------------------------------------------------------------
def _build_program(ed):
    import concourse.bass as bass
    import concourse.mybir as mybir
    import concourse.tile as tile
    from concourse import bacc

    T, maxt = ed["T"], ed["maxt"]
    ntiles = ed["ntiles"]
    toff = np.concatenate([[0], np.cumsum(ntiles)]).astype(int)
    f32 = mybir.dt.float32
    bf16 = mybir.dt.bfloat16
    i16 = mybir.dt.int16
    Alu = mybir.AluOpType
    Act = mybir.ActivationFunctionType

    nc = bacc.Bacc("TRN2", target_bir_lowering=False, debug=False,
                   enable_asserts=False, num_devices=NCORES)

    xT_d = nc.dram_tensor("xT", [F, RC * N], bf16, kind="ExternalInput").ap()
    waug_d = nc.dram_tensor("w_aug", [F, 264], bf16, kind="ExternalInput").ap()
    oh_d = nc.dram_tensor("oh", [128, T * DTW], bf16, kind="ExternalInput").ap()
    ohT_d = nc.dram_tensor("ohT", [128, T * 128], bf16, kind="ExternalInput").ap()
    ih_d = nc.dram_tensor("ih", [128, T * 8], i16, kind="ExternalInput").ap()
    bias_d = nc.dram_tensor("bias_slab", [128, RC * F], f32, kind="ExternalInput").ap()
    out_d = nc.dram_tensor("out", [RC, N, F], f32, kind="ExternalOutput").ap()

    with tile.TileContext(nc) as tc:
        with (
            tc.tile_pool(name="const", bufs=1) as constp,
            tc.tile_pool(name="dram", bufs=1, space="DRAM") as dramp,
            tc.tile_pool(name="stage", bufs=3) as stagep,
            tc.tile_pool(name="edge", bufs=2) as edgep,
            tc.tile_pool(name="big", bufs=2) as bigp,
            tc.tile_pool(name="fin", bufs=2) as finp,
            tc.tile_pool(name="spsum", bufs=2, space="PSUM") as spsum,
            tc.tile_pool(name="npsum", bufs=2, space="PSUM") as npsum,
        ):
            h_hbm = dramp.tile([N + 1, ROWW], bf16)

            # ---- constants needed before/during phase A ----
            waug = constp.tile([F, 264], bf16)
            nc.sync.dma_start(waug[:], waug_d)

            # pad row 1000: h-part zeros, as-part -1000 => p == 0 for pad slots
            padrow = constp.tile([1, ROWW], bf16)
            nc.vector.memset(padrow[:], 0.0)
            nc.vector.memset(padrow[:, RC * HC:RC * HC + AC], -1000.0)
            nc.sync.dma_start(h_hbm[N:N + 1, :], padrow[:])

            # ---- phase A: projection; fills h_hbm + hslab (h|as|ad rows) ----
            hslab = constp.tile([128, RC, NDT, 264], bf16)
            # eps reads ad cols of tile 7 rows 104.. -> must not be NaN
            nc.vector.memset(hslab[96:128, :, 7, 260:264], 0.0)
            for r in range(RC):
                xt = stagep.tile([F, N], bf16, tag="xt")
                nc.sync.dma_start(xt[:], xT_d[:, r * N:(r + 1) * N])
                for nt in range(NDT):
                    n0 = nt * DTW
                    w = _tile_w(nt)
                    ps = spsum.tile([128, maxt * AC], f32, tag="scratch")
                    nc.tensor.matmul(out=ps[0:w, 0:264], lhsT=xt[:, n0:n0 + w],
                                     rhs=waug[:], start=True, stop=True)
                    nc.scalar.copy(out=hslab[0:w, r, nt, 0:128],
                                   in_=ps[0:w, 0:128])
                    nc.vector.tensor_copy(out=hslab[0:w, r, nt, 128:264],
                                          in_=ps[0:w, 128:264])
                nc.sync.dma_start(
                    h_hbm[0:896, r * HC:(r + 1) * HC].rearrange(
                        "(a d) e -> d a e", d=DTW), hslab[:, r, 0:7, 0:256])
                nc.sync.dma_start(h_hbm[896:1000, r * HC:(r + 1) * HC],
                                  hslab[0:104, r, 7, 0:256])
                c0 = RC * HC + 4 * r
                nc.sync.dma_start(
                    h_hbm[0:896, c0:c0 + 4].rearrange(
                        "(a d) e -> d a e", d=DTW), hslab[:, r, 0:7, 256:260])
                nc.sync.dma_start(h_hbm[896:1000, c0:c0 + 4],
                                  hslab[0:104, r, 7, 256:260])

            # ---- edge-phase constants (after phase A's DMAs in queue order)
            ih = constp.tile([128, T * 8], i16)
            nc.sync.dma_start(ih[:], ih_d)
            ohT = constp.tile([128, T, 128], bf16)
            nc.sync.dma_start(ohT[:], ohT_d.rearrange("p (t e) -> p t e", e=128))
            oh = constp.tile([128, T, DTW], bf16)
            nc.sync.dma_start(oh[:], oh_d.rearrange("p (t d) -> p t d", d=DTW))
            bias_sl = constp.tile([128, RC, F], f32)
            nc.sync.dma_start(bias_sl[:], bias_d.rearrange("p (r f) -> p r f", f=F))

            # ---- edge phase, software-pipelined over the 8 dst-tile chunks --
            def gather(t):
                nt_ = ntiles[t]
                ni = nt_ * 128
                assert ni <= 2032, "gather exceeds SWDGE FIFO"
                hg = bigp.tile([128, maxt, ROWW], bf16, tag="big")
                nc.gpsimd.dma_gather(
                    out_ap=hg[:, 0:nt_, :], in_ap=h_hbm[:],
                    idxs_ap=ih[:, toff[t] * 8:(toff[t] + nt_) * 8],
                    num_idxs=ni, num_idxs_reg=ni, elem_size=ROWW,
                    single_packet=False)
                return hg

            def eps_mm(t):
                nt_ = ntiles[t]
                eps = spsum.tile([128, maxt * AC], f32, tag="scratch")
                for k in range(nt_):
                    nc.tensor.matmul(out=eps[:, k * AC:(k + 1) * AC],
                                     lhsT=ohT[:, toff[t] + k, :],
                                     rhs=hslab[:, :, t, 260:264],
                                     start=True, stop=True)
                return eps

            def attn_prep(t, hg, eps):
                nt_ = ntiles[t]
                z = edgep.tile([128, maxt, AC], bf16, tag="z")
                nc.vector.tensor_tensor(
                    out=z[:, 0:nt_, :],
                    in0=hg[:, 0:nt_, RC * HC:RC * HC + AC],
                    in1=eps[:, 0:nt_ * AC].rearrange("p (t a) -> p t a", a=AC),
                    op=Alu.add)
                nc.vector.scalar_tensor_tensor(
                    out=z[:, 0:nt_, :], in0=z[:, 0:nt_, :], scalar=NEG_SLOPE,
                    in1=z[:, 0:nt_, :], op0=Alu.mult, op1=Alu.max)
                p_bf = edgep.tile([128, maxt, AC], bf16, tag="p")
                nc.scalar.activation(out=p_bf[:, 0:nt_, :], in_=z[:, 0:nt_, :],
                                     func=Act.Exp)
                return p_bf

            def den_mm_pe(t, p_bf):
                nt_ = ntiles[t]
                den_ps = spsum.tile([128, maxt * AC], f32, tag="scratch")
                for k in range(nt_):
                    nc.tensor.matmul(out=den_ps[:, 0:AC],
                                     lhsT=oh[:, toff[t] + k, :],
                                     rhs=p_bf[:, k, :],
                                     start=(k == 0), stop=(k == nt_ - 1))
                return den_ps

            def den_recip(den_ps):
                denrec = stagep.tile([128, AC], f32, tag="denrec")
                nc.vector.reciprocal(out=denrec[:], in_=den_ps[:, 0:AC])
                nc.vector.tensor_scalar_mul(denrec[:], denrec[:], 0.25)
                return denrec

            def mult_msg(t, hg, p_bf):
                nt_ = ntiles[t]
                for r in range(RC):
                    hgr = hg[:, 0:nt_, r * HC:(r + 1) * HC].rearrange(
                        "p t (c h) -> p t c h", h=H)
                    pb = p_bf[:, 0:nt_, 4 * r:4 * r + 4].rearrange(
                        "p t (o h) -> p t o h", o=1).to_broadcast(
                        [128, nt_, C, H])
                    nc.vector.tensor_tensor(out=hgr, in0=hgr, in1=pb,
                                            op=Alu.mult)

            def num_mm(t, hg):
                nt_ = ntiles[t]
                nps = npsum.tile([128, RC * HC], f32, tag="num")
                for k in range(nt_):
                    lhsT = oh[:, toff[t] + k, :]
                    for g in range(3):
                        nc.tensor.matmul(out=nps[:, g * 512:(g + 1) * 512],
                                         lhsT=lhsT,
                                         rhs=hg[:, k, g * 512:(g + 1) * 512],
                                         start=(k == 0), stop=(k == nt_ - 1))
                return nps

            def finalize(t, nps, denrec):
                n0 = t * DTW
                w = _tile_w(t)
                for half in range(2):
                    c0 = half * (RC * HC // 2)
                    numn = finp.tile([128, RC // 2, HC], f32, tag="numn")
                    drb = denrec[:, half * 12:half * 12 + 12].rearrange(
                        "d (r o h) -> d r o h", h=H, o=1).to_broadcast(
                        [128, RC // 2, C, H])
                    nc.vector.tensor_tensor(
                        out=numn[:].rearrange("d r (c h) -> d r c h", h=H),
                        in0=nps[:, c0:c0 + 768].rearrange(
                            "d (r c h) -> d r c h", h=H, c=C),
                        in1=drb, op=Alu.mult)
                    n4 = numn[:].rearrange("d r (c h) -> d r c h", h=H)
                    t12 = finp.tile([128, RC // 2, C, 2], f32, tag="t12")
                    s = finp.tile([128, RC // 2, C], f32, tag="s")
                    ob = finp.tile([128, RC // 2, C], f32, tag="ob")
                    nc.vector.tensor_tensor(out=t12[:], in0=n4[:, :, :, 0:2],
                                            in1=n4[:, :, :, 2:4], op=Alu.add)
                    nc.vector.tensor_tensor(out=s[:], in0=t12[:, :, :, 0],
                                            in1=t12[:, :, :, 1], op=Alu.add)
                    nc.vector.tensor_tensor(
                        out=ob[:], in0=s[:],
                        in1=bias_sl[:, half * 3:half * 3 + 3, :], op=Alu.add)
                    nc.sync.dma_start(
                        out_d[half * 3:half * 3 + 3, n0:n0 + w, :].rearrange(
                            "r d f -> d r f"), ob[0:w])

            # prologue: chunk 0 attn-prep, plus chunk 1 gather/eps in flight
            hgs, epss, pbs, drs = {}, {}, {}, {}
            hgs[0] = gather(0)
            epss[0] = eps_mm(0)
            pbs[0] = attn_prep(0, hgs[0], epss[0])
            dp0 = den_mm_pe(0, pbs[0])
            hgs[1] = gather(1)
            epss[1] = eps_mm(1)
            drs[0] = den_recip(dp0)
            for t in range(NDT):
                mult_msg(t, hgs[t], pbs[t])
                nps = num_mm(t, hgs[t])
                if t + 1 < NDT:
                    pbs[t + 1] = attn_prep(t + 1, hgs[t + 1], epss[t + 1])
                    dp = den_mm_pe(t + 1, pbs[t + 1])
                if t + 2 < NDT:
                    hgs[t + 2] = gather(t + 2)
                    epss[t + 2] = eps_mm(t + 2)
                finalize(t, nps, drs[t])
                if t + 1 < NDT:
                    drs[t + 1] = den_recip(dp)

    nc.compile()
    return nc


# --------------------------------------------------------------------------
# public entry point
# --------------------------------------------------------------------------
def kernel(x, edge_index, W, att_src, att_dst, bias):
    key = hash(np.asarray(edge_index).tobytes())
    if key not in _CACHE:
        ed = _prep_edges(edge_index)
        _CACHE[key] = (_build_program(ed), ed)
    nc, ed = _CACHE[key]

    in_maps = _make_in_maps(x, W, att_src, att_dst, bias, ed)
    from concourse import bass_utils
    res = bass_utils.run_bass_kernel_spmd(nc, in_maps, core_ids=list(range(NCORES)))
    outs = [res.results[c]["out"] for c in range(NCORES)]
    out = np.concatenate(outs, axis=0).reshape(B, S, N, F).astype(np.float32)
    return out
